# revision 1
# baseline (speedup 1.0000x reference)
"""Trainium2 Bass kernel for nn_DeforConv_71605694759687.

ResBlock(stride2, 64->128) + DCNv2 (modulated deformable conv) + BN + ReLU.

Sharding (8 cores): (batch b = core//4, H-quarter q = core%4); each core
computes 32 output rows of out[b] end-to-end locally (halo via recompute,
no collectives).

Deformable sampling is gather-free: bilinear sampling at (base + k + d),
|d| < 2, is expanded over a 5x5 window of static shifts j with tent
coefficients t_j = relu(1 - |d - j|) (exactly the bilinear weights;
self-pruning to zero outside the active 2x2 cell).  The mask*ty*tx
coefficient maps are partition-broadcast across the 64 channels of each
deform group via tiny K=2 selector matmuls on the PE, and the rhs of the
main einsum is the Hadamard product C_bcast * shifted-feat-view (DVE,
bf16), contracted on the PE over (k, sigma, d, c).
"""

import numpy as np
import ml_dtypes
from contextlib import ExitStack

import concourse.bass as bass
import concourse.tile as tile
from concourse import mybir, bacc
from concourse.bass_utils import run_bass_kernel_spmd

F32 = mybir.dt.float32
BF16 = mybir.dt.bfloat16
AL = mybir.AluOpType
AF = mybir.ActivationFunctionType

P = 128
EPS = 1e-5
Ci, Co, DG, Cg = 64, 128, 2, 64
H, W = 128, 128          # output spatial (after stride-2)
QROWS = 32               # output rows per core
JW = (-2, -1, 0, 1, 2)   # tent window per axis (exact for |offset| < 2)
NSIG = 25
FR, FC = 38, 134         # F_pad: rows h0-3..h0+34, cols x in [-3,130]
F1R, F1C = 40, 130       # feat1: rows h0-4..h0+35, cols [-1,128]
XR, XC = 81, 258         # x_pad: rows 2*h0-9..2*h0+71, cols [-1,256]
NCHUNK = 1024
NPC = 4


def _bf(x):
    return np.ascontiguousarray(x.astype(ml_dtypes.bfloat16))


def _f(x):
    return np.ascontiguousarray(np.asarray(x, dtype=np.float32))


def build_nc():
    nc = bacc.Bacc(None)

    d_x = nc.dram_tensor("x_shard", [Ci, XR, XC], F32, kind="ExternalInput")
    d_l1 = nc.dram_tensor("lhsT1", [Ci, 9, P], F32, kind="ExternalInput")
    d_l2 = nc.dram_tensor("lhsT2", [P, 9, P], F32, kind="ExternalInput")
    d_lsc = nc.dram_tensor("lhsT_sc", [Ci, P], F32, kind="ExternalInput")
    d_loff = nc.dram_tensor("lhsT_off", [P, 9, 54], F32, kind="ExternalInput")
    d_ldcn = nc.dram_tensor("lhsT_dcn", [P, 9, P], BF16, kind="ExternalInput")
    d_esel = nc.dram_tensor("e_sel", [P, 9, P], BF16, kind="ExternalInput")
    d_cst = nc.dram_tensor("consts", [P, 16], F32, kind="ExternalInput")
    d_bq = nc.dram_tensor("bias_q", [P, 3], F32, kind="ExternalInput")
    d_rm1 = nc.dram_tensor("rowmask1", [P, F1R], F32, kind="ExternalInput")
    d_rmf = nc.dram_tensor("rowmaskF", [P, FR], F32, kind="ExternalInput")
    d_out = nc.dram_tensor("out", [P, QROWS, W], F32, kind="ExternalOutput")

    with tile.TileContext(nc) as tc, ExitStack() as ctx:
        singles = ctx.enter_context(tc.tile_pool(name="singles", bufs=1))

        # ---- persistent SBUF ----
        fpadA = singles.tile([P, FR, FC], F32)      # col c <-> x-3
        fA = singles.tile([P, FR, FC], BF16)        # bf16, col c <-> x-3
        fB = singles.tile([P, FR, FC], BF16)        # bf16, col c <-> x-2
        ldcn = singles.tile([P, 9, P], BF16)
        esel = singles.tile([P, 9, P], BF16)
        cst = singles.tile([P, 16], F32)

        nc.sync.dma_start(out=ldcn[:], in_=d_ldcn[:])
        nc.sync.dma_start(out=esel[:], in_=d_esel[:])
        nc.sync.dma_start(out=cst[:], in_=d_cst[:])

        inv1, beta1 = cst[:, 0:1], cst[:, 1:2]
        inv2, beta2 = cst[:, 2:3], cst[:, 3:4]
        inv3, beta3 = cst[:, 4:5], cst[:, 5:6]

        nc.vector.memset(fpadA[:, :, 0:3], 0.0)
        nc.vector.memset(fpadA[:, :, FC - 3:FC], 0.0)

        # ================= Phase A: ResBlock =================
        with tc.tile_pool(name="ph_a", bufs=1) as pa, \
             tc.tile_pool(name="psum_a", bufs=2, space="PSUM") as psa:
            x_pad = pa.tile([Ci, XR, XC], F32)
            feat1 = pa.tile([P, F1R, F1C], F32)
            l1 = pa.tile([Ci, 9, P], F32)
            l2 = pa.tile([P, 9, P], F32)
            lsc = pa.tile([Ci, P], F32)
            rm1 = pa.tile([P, F1R], F32)
            rmf = pa.tile([P, FR], F32)
            for i in range(8):
                r0, r1 = (XR * i) // 8, (XR * (i + 1)) // 8
                nc.sync.dma_start(out=x_pad[:, r0:r1, :],
                                  in_=d_x[:, r0:r1, :])
            for t, dref in ((l1, d_l1), (l2, d_l2),
                            (lsc, d_lsc), (rm1, d_rm1), (rmf, d_rmf)):
                nc.sync.dma_start(out=t[:], in_=dref[:])

            nc.vector.memset(feat1[:, :, 0:1], 0.0)
            nc.vector.memset(feat1[:, :, F1C - 1:F1C], 0.0)

            # conv1 3x3 s2 + bn1 + relu -> feat1
            # feat1 row f1 <-> global h0-4+f1; reads x_pad rows 2*f1+ty,
            # cols 2*c+tx; writes cols 1..128
            for cki in range(10):
                r0 = cki * 4
                ps = psa.tile([P, 4, W], F32)
                for t in range(9):
                    ty, tx = t // 3, t % 3
                    rhs = x_pad[:, 2 * r0 + ty: 2 * r0 + ty + 7: 2,
                                tx: tx + 2 * W - 1: 2]
                    nc.tensor.matmul(ps[:], l1[:, t, :], rhs,
                                     start=(t == 0), stop=(t == 8))
                nc.scalar.activation(feat1[:, r0:r0 + 4, 1:1 + W], ps[:],
                                     AF.Relu, bias=beta1, scale=inv1)
            nc.vector.tensor_tensor(
                feat1[:], feat1[:],
                rm1[:, :, None].to_broadcast(feat1.shape), AL.mult)

            # conv2 3x3 s1 (+ folded shortcut) + bn + relu -> fpadA
            # fpad row f2 <-> global h0-3+f2; feat1 rows f2+ty cols c+tx;
            # shortcut x_pad rows 2*f2+3, cols 2*c+1; writes cols 3..130
            for cki in range(10):
                r0 = cki * 4
                nrow = min(4, FR - r0)
                ps = psa.tile([P, 4, W], F32, tag="ps2")
                for t in range(9):
                    ty, tx = t // 3, t % 3
                    rhs = feat1[:, r0 + ty: r0 + ty + nrow, tx: tx + W]
                    nc.tensor.matmul(ps[:, :nrow], l2[:, t, :], rhs,
                                     start=(t == 0), stop=False)
                rhs_sc = x_pad[:, 2 * r0 + 3: 2 * r0 + 2 + 2 * nrow: 2,
                               1: 2 * W: 2]
                nc.tensor.matmul(ps[:, :nrow], lsc[:], rhs_sc,
                                 start=False, stop=True)
                nc.scalar.activation(fpadA[:, r0:r0 + nrow, 3:3 + W],
                                     ps[:, :nrow], AF.Relu,
                                     bias=beta2, scale=inv2)
            nc.vector.tensor_tensor(
                fpadA[:], fpadA[:],
                rmf[:, :, None].to_broadcast(fpadA.shape), AL.mult)

        late = ctx.enter_context(tc.tile_pool(name="late", bufs=1))
        c_all = late.tile([P, NSIG, NCHUNK], BF16)  # rows pq*32+2k+d

        nc.vector.tensor_copy(out=fA[:], in_=fpadA[:])
        nc.vector.tensor_copy(out=fB[:, :, 0:FC - 1], in_=fpadA[:, :, 1:FC])
        nc.vector.memset(fB[:, :, FC - 1:FC], 0.0)

        # ================= Phase B: offsets -> coefficients =================
        with tc.tile_pool(name="ph_b", bufs=1) as pb, \
             tc.tile_pool(name="ph_b_tmp", bufs=2) as pbt, \
             tc.tile_pool(name="psum_b", bufs=2, space="PSUM") as psb:
            loff = pb.tile([P, 9, 54], F32)
            bq = pb.tile([P, 3], F32)
            q_t = pb.tile([P, 3, NCHUNK], F32)     # dy, dx, mm
            mask_t = pb.tile([P, NCHUNK], BF16)
            ty_t = pb.tile([P, 5, NCHUNK], BF16)
            tx_t = pb.tile([P, 5, NCHUNK], BF16)
            nc.sync.dma_start(out=loff[:], in_=d_loff[:])
            nc.sync.dma_start(out=bq[:], in_=d_bq[:])

            # offset conv om: rows quant*18+k*2+d; out rows h0..h0+31
            # out row r: fpad rows r+2+ty, cols c+2+tx
            for cki in range(8):
                r0 = cki * 4
                pq, c2 = cki // 2, cki % 2
                ps = psb.tile([54, 4, W], F32)
                for t in range(9):
                    ty, tx = t // 3, t % 3
                    rhs = fpadA[:, r0 + 2 + ty: r0 + 6 + ty, 2 + tx: 2 + tx + W]
                    nc.tensor.matmul(ps[:], loff[:, t, :], rhs,
                                     start=(t == 0), stop=(t == 8))
                om_sb = pbt.tile([54, 512], F32, tag="om_sb")
                nc.scalar.copy(om_sb[:], ps[:].rearrange("p a b -> p (a b)"))
                for q in range(3):
                    nc.gpsimd.dma_start(
                        out=q_t[pq * 32: pq * 32 + 18, q,
                                c2 * 512:(c2 + 1) * 512],
                        in_=om_sb[q * 18:(q + 1) * 18, :])

            nc.vector.tensor_tensor(
                q_t[:], q_t[:], bq[:, :, None].to_broadcast(q_t.shape),
                AL.add)
            nc.scalar.activation(mask_t[:], q_t[:, 2, :], AF.Sigmoid)
            # tents t_j = relu(1 - |d - j|)
            for ax, dst in ((0, ty_t), (1, tx_t)):
                for ji, j in enumerate(JW):
                    a = pbt.tile([P, NCHUNK], F32, tag="tent_tmp")
                    nc.scalar.activation(a[:], q_t[:, ax, :], AF.Abs,
                                         bias=cst[:, 8 + ji:9 + ji])
                    nc.vector.tensor_scalar(a[:], a[:], -1.0, 1.0,
                                            AL.mult, AL.add)
                    nc.vector.tensor_scalar(dst[:, ji, :], a[:], 0.0, None,
                                            AL.max)
            # C = mask * ty * tx  -> bf16
            for ji in range(5):
                a = pbt.tile([P, NCHUNK], BF16, tag="prod_tmp")
                nc.vector.tensor_tensor(a[:], mask_t[:], ty_t[:, ji, :],
                                        AL.mult)
                for xi in range(5):
                    nc.vector.tensor_tensor(c_all[:, ji * 5 + xi, :], a[:],
                                            tx_t[:, xi, :], AL.mult)

        # ================= Phase C: bcast + Hadamard + einsum =================
        # p-chunks processed in pairs; the two chunks' selector matmuls sit in
        # adjacent 32-row groups (tile_position) so the PE overlaps them.
        with tc.tile_pool(name="psum_cb", bufs=1, space="PSUM") as psum_cb, \
             tc.tile_pool(name="psum_out", bufs=1, space="PSUM") as psum_out, \
             tc.tile_pool(name="cbs", bufs=4) as cbs_pool, \
             tc.tile_pool(name="rhs", bufs=4) as rhs_pool, \
             tc.tile_pool(name="outs", bufs=2) as out_pool:
            for pp in range(2):
                pos = [psum_out.tile([P, NCHUNK], F32, name=f"po{h}", tag=f"po{h}")
                       for h in range(2)]
                nj = 0
                for k in range(9):
                    ky, kx = k // 3, k % 3
                    for ji, jy in enumerate(JW):
                        for xi, jx in enumerate(JW):
                            sig = ji * 5 + xi
                            sy, sx = ky - 1 + jy, kx - 1 + jx
                            for h in range(2):
                                pc = pp * 2 + h
                                yb = pc * 8
                                rowp = pc * 32
                                cb = psum_cb.tile([P, NCHUNK], F32,
                                                  name=f"cb{h}", tag=f"cb{h}")
                                for h5 in range(2):
                                    nc.tensor.matmul(
                                        cb[:, h5 * 512:(h5 + 1) * 512],
                                        esel[rowp: rowp + 18, k, :],
                                        c_all[rowp: rowp + 18, sig,
                                              h5 * 512:(h5 + 1) * 512],
                                        start=True, stop=True,
                                        tile_position=(rowp, 0))
                                cbs = cbs_pool.tile([P, NCHUNK], BF16)
                                nc.scalar.copy(cbs[:], cb[:])
                                if (3 + sx) % 2 == 0:
                                    fv = fA[:, yb + 3 + sy: yb + 11 + sy,
                                            3 + sx: 131 + sx]
                                else:
                                    fv = fB[:, yb + 3 + sy: yb + 11 + sy,
                                            2 + sx: 130 + sx]
                                rt = rhs_pool.tile([P, NCHUNK], BF16)
                                nc.vector.tensor_tensor(
                                    rt[:].rearrange("p (a b) -> p a b", a=8),
                                    cbs[:].rearrange("p (a b) -> p a b", a=8),
                                    fv, AL.mult)
                                for h5 in range(2):
                                    nc.tensor.matmul(
                                        pos[h][:, h5 * 512:(h5 + 1) * 512],
                                        ldcn[:, k, :],
                                        rt[:, h5 * 512:(h5 + 1) * 512],
                                        start=(nj == 0),
                                        stop=(nj == 9 * NSIG - 1))
                            nj += 1
                for h in range(2):
                    pc = pp * 2 + h
                    yb = pc * 8
                    ob = out_pool.tile([P, NCHUNK], F32)
                    nc.scalar.activation(ob[:], pos[h][:], AF.Relu,
                                         bias=beta3, scale=inv3)
                    nc.sync.dma_start(
                        out=d_out[:, yb: yb + 8, :],
                        in_=ob[:].rearrange("p (a b) -> p a b", a=8))

    nc.compile()
    return nc


_CACHE = {}


def _prep(inputs):
    f = {k: _f(v) for k, v in inputs.items()}
    inv1 = f['g1'] / np.sqrt(f['v1'] + EPS)
    beta1 = f['b1'] - f['m1'] * inv1
    inv2 = f['g2'] / np.sqrt(f['v2'] + EPS)
    beta2 = f['b2'] - f['m2'] * inv2
    invd = f['gd'] / np.sqrt(f['vd'] + EPS)
    betad = f['bd'] - f['md'] * invd
    inv3 = f['g3'] / np.sqrt(f['v3'] + EPS)
    beta3 = f['b3'] - f['m3'] * inv3

    lhsT1 = np.transpose(f['w1'], (1, 2, 3, 0)).reshape(Ci, 9, P)
    lhsT2 = np.transpose(f['w2'], (1, 2, 3, 0)).reshape(P, 9, P)
    wd = f['wd'][:, :, 0, 0] * (invd / inv2)[:, None]
    lhsT_sc = np.ascontiguousarray(wd.T)

    # offset conv rows: quant*18 + k*2 + d  <-  orig quant*18 + d*9 + k
    perm = np.zeros(54, dtype=np.int64)
    for quant in range(3):
        for kk in range(9):
            for dd in range(2):
                perm[quant * 18 + kk * 2 + dd] = quant * 18 + dd * 9 + kk
    ow = f['off_w'][perm]
    obias = f['off_b'][perm]
    lhsT_off = np.transpose(ow, (1, 2, 3, 0)).reshape(P, 9, 54)

    wr = f['dcn_w'].reshape(Co, DG, Cg, 9)
    lhsT_dcn = np.transpose(wr, (1, 2, 3, 0)).reshape(P, 9, Co)

    esel = np.zeros((P, 9, P), dtype=np.float32)
    for s in range(4):
        for kk in range(9):
            for dd in range(2):
                esel[32 * s + 2 * kk + dd, kk, dd * 64:(dd + 1) * 64] = 1.0

    cst = np.zeros((P, 16), dtype=np.float32)
    for ji, j in enumerate((-2, -1, 0, 1, 2)):
        cst[:, 8 + ji] = -float(j)
    cst[:, 0], cst[:, 1] = inv1, beta1
    cst[:, 2], cst[:, 3] = inv2, beta2 + betad
    cst[:, 4], cst[:, 5] = inv3, beta3 + inv3 * f['dcn_b']

    bias_q = np.zeros((P, 3), dtype=np.float32)
    for pq in range(4):
        for kk in range(9):
            for dd in range(2):
                r = pq * 32 + kk * 2 + dd
                for quant in range(3):
                    bias_q[r, quant] = obias[quant * 18 + kk * 2 + dd]

    return dict(
        lhsT1=_f(lhsT1), lhsT2=_f(lhsT2), lhsT_sc=_f(lhsT_sc),
        lhsT_off=_f(lhsT_off), lhsT_dcn=_bf(lhsT_dcn), e_sel=_bf(esel),
        consts=_f(cst), bias_q=_f(bias_q), x=f['x'])


def kernel(**inputs):
    cfg = _prep(inputs)
    x = cfg.pop('x')
    B = x.shape[0]

    if 'nc' not in _CACHE:
        _CACHE['nc'] = build_nc()
    nc = _CACHE['nc']

    in_maps = []
    for cid in range(8):
        b, q = cid // 4, cid % 4
        h0 = 32 * q
        xp = np.zeros((Ci, XR, XC), dtype=np.float32)
        r_lo = 2 * h0 - 9
        s_lo, s_hi = max(r_lo, 0), min(2 * h0 + 72, 256)
        xp[:, s_lo - r_lo: s_hi - r_lo, 1:257] = x[b, :, s_lo:s_hi, :]
        rm1 = np.zeros((P, F1R), dtype=np.float32)
        for f1 in range(F1R):
            rm1[:, f1] = 1.0 if 0 <= h0 - 4 + f1 < H else 0.0
        rmf = np.zeros((P, FR), dtype=np.float32)
        for f2 in range(FR):
            rmf[:, f2] = 1.0 if 0 <= h0 - 3 + f2 < H else 0.0
        m = dict(cfg)
        m['x_shard'] = np.ascontiguousarray(xp)
        m['rowmask1'] = rm1
        m['rowmaskF'] = rmf
        in_maps.append(m)

    res = run_bass_kernel_spmd(nc, in_maps, core_ids=list(range(8)))
    out = np.zeros((B, Co, H, W), dtype=np.float32)
    for cid in range(8):
        b, q = cid // 4, cid % 4
        out[b, :, 32 * q:32 * q + 32, :] = res.results[cid]['out']
    return out



# revision 6
# speedup vs baseline: 2.7321x; 2.7321x over previous
"""Trainium2 Bass kernel for nn_DeforConv_71605694759687 (gather-based).

ResBlock(stride2, 64->128) + DCNv2 (modulated deformable conv) + BN + ReLU.

Sharding (8 cores): (batch b = core//4, H-quarter q = core%4); each core
computes 32 output rows of out[b] end-to-end locally (halo via recompute,
no collectives).

Unlike the tent-expansion design, deformable sampling here uses real
GPSIMD gathers (ap_gather): per 3x3 tap k, the four bilinear corner
values are gathered from the padded feature map at runtime-computed
int16 indices, multiplied by per-corner coefficient maps
(mask * bilinear weights, broadcast from 18 rows to 128 partitions via
DRAM-bounce replication DMAs), and contracted on the PE with the DCN
weights (9 taps x 4 corners accumulating matmuls).

Spatial positions use the "i-order" i = ((y%2)*128 + x)*16 + y//2 so
that the ap_gather 16-partition index wrap, the offset-conv rhs view,
and the output un-permute are all regular strided APs.
"""

import numpy as np
import ml_dtypes
from contextlib import ExitStack

import concourse.bass as bass
import concourse.tile as tile
from concourse import mybir, bacc
from concourse.bass_utils import run_bass_kernel_spmd

F32 = mybir.dt.float32
F32R = mybir.dt.float32r
BF16 = mybir.dt.bfloat16
I16 = mybir.dt.int16
AL = mybir.AluOpType
AF = mybir.ActivationFunctionType

P = 128
EPS = 1e-5
Ci, Co, DG, Cg = 64, 128, 2, 64
H, W = 128, 128          # output spatial (after stride-2)
QROWS = 32               # output rows per core
FR = 40                  # F_pad rows: h0-3 .. h0+34 (+2 zero guard rows)
FC = 134                 # F_pad cols: x in [-3, 130]
NELEM = FR * FC          # 5360 gather elements per partition
F1R, F1C = 40, 130       # feat1: rows h0-4..h0+35, cols [-1,128]
XR, XC = 81, 258         # x_pad: rows 2*h0-9..2*h0+71, cols [-1,256]
S = 4096                 # spatial positions per core (32*128)
MAGIC = 12582912.0       # 1.5 * 2^23, fp32 RNE rounding trick
IDXMAX = 37 * FC + 132   # max legal idx00


def _bf(x):
    return np.ascontiguousarray(np.asarray(x).astype(ml_dtypes.bfloat16))


def _f(x):
    return np.ascontiguousarray(np.asarray(x, dtype=np.float32))


def build_nc():
    nc = bacc.Bacc(None)

    d_x = nc.dram_tensor("x_shard", [Ci, XR, XC], F32R, kind="ExternalInput")
    d_l1 = nc.dram_tensor("lhsT1", [Ci, 9, P], F32R, kind="ExternalInput")
    d_l2 = nc.dram_tensor("lhsT2", [P, 9, P], BF16, kind="ExternalInput")
    d_lsc = nc.dram_tensor("lhsT_sc", [Ci, P], F32R, kind="ExternalInput")
    d_loff = nc.dram_tensor("lhsT_off", [P, 9, 54], BF16, kind="ExternalInput")
    d_ldcn = nc.dram_tensor("lhsT_dcn", [P, 9, P], BF16, kind="ExternalInput")
    d_cst = nc.dram_tensor("consts", [P, 10], F32, kind="ExternalInput")
    d_yadd = nc.dram_tensor("y_add", [18, S], F32, kind="ExternalInput")
    d_xadd = nc.dram_tensor("x_add", [18, S], F32, kind="ExternalInput")
    d_rm1 = nc.dram_tensor("rowmask1", [P, F1R], F32, kind="ExternalInput")
    d_rmf = nc.dram_tensor("rowmaskF", [P, FR], F32, kind="ExternalInput")
    d_out = nc.dram_tensor("out", [P, QROWS, W], F32, kind="ExternalOutput")

    d_cm = nc.dram_tensor("cmaps", [4, 18, S], BF16, kind="Internal")
    d_iw = nc.dram_tensor("idxw", [18, 16, 256], I16, kind="Internal")

    with tile.TileContext(nc) as tc, ExitStack() as ctx:
        singles = ctx.enter_context(tc.tile_pool(name="singles", bufs=1))

        fpadA = singles.tile([P, FR, FC], F32)     # gather source, col c <-> x-3
        ldcn = singles.tile([P, 9, P], BF16)
        cst = singles.tile([P, 10], F32)
        nc.sync.dma_start(out=ldcn[:], in_=d_ldcn[:])
        nc.sync.dma_start(out=cst[:], in_=d_cst[:])

        inv1, beta1 = cst[:, 0:1], cst[:, 1:2]
        inv2, beta2 = cst[:, 2:3], cst[:, 3:4]
        inv3, beta3 = cst[:, 4:5], cst[:, 5:6]

        nc.vector.memset(fpadA[:, :, 0:3], 0.0)
        nc.vector.memset(fpadA[:, :, FC - 3:FC], 0.0)
        nc.vector.memset(fpadA[:, 38:40, :], 0.0)

        # ================= Phase A: ResBlock =================
        with tc.tile_pool(name="ph_a", bufs=1) as pa, \
             tc.tile_pool(name="psum_a", bufs=2, space="PSUM") as psa:
            x_pad = pa.tile([Ci, XR, XC], F32R)
            feat1 = pa.tile([P, F1R, F1C], BF16)
            l1 = pa.tile([Ci, 9, P], F32R)
            l2 = pa.tile([P, 9, P], BF16)
            lsc = pa.tile([Ci, P], F32R)
            rm1 = pa.tile([P, F1R], F32)
            rmf = pa.tile([P, FR], F32)
            for i in range(8):
                r0, r1 = (XR * i) // 8, (XR * (i + 1)) // 8
                nc.sync.dma_start(out=x_pad[:, r0:r1, :],
                                  in_=d_x[:, r0:r1, :])
            for t, dref in ((l1, d_l1), (l2, d_l2),
                            (lsc, d_lsc), (rm1, d_rm1), (rmf, d_rmf)):
                nc.sync.dma_start(out=t[:], in_=dref[:])

            nc.vector.memset(feat1[:, :, 0:1], 0.0)
            nc.vector.memset(feat1[:, :, F1C - 1:F1C], 0.0)

            # conv1 3x3 s2 + bn1 + relu -> feat1 (bf16)
            for cki in range(10):
                r0 = cki * 4
                ps = psa.tile([P, 4, W], F32)
                for t in range(9):
                    ty, tx = t // 3, t % 3
                    rhs = x_pad[:, 2 * r0 + ty: 2 * r0 + ty + 7: 2,
                                tx: tx + 2 * W - 1: 2]
                    nc.tensor.matmul(ps[:], l1[:, t, :], rhs,
                                     start=(t == 0), stop=(t == 8))
                nc.scalar.activation(feat1[:, r0:r0 + 4, 1:1 + W], ps[:],
                                     AF.Relu, bias=beta1, scale=inv1)
            nc.vector.tensor_tensor(
                feat1[:], feat1[:],
                rm1[:, :, None].to_broadcast(feat1.shape), AL.mult)

            # conv2 3x3 s1 (+ folded shortcut) + bn + relu -> fpadA rows 0..37
            for cki in range(10):
                r0 = cki * 4
                nrow = min(4, 38 - r0)
                ps = psa.tile([P, 4, W], F32, tag="ps2")
                for t in range(9):
                    ty, tx = t // 3, t % 3
                    rhs = feat1[:, r0 + ty: r0 + ty + nrow, tx: tx + W]
                    nc.tensor.matmul(ps[:, :nrow], l2[:, t, :], rhs,
                                     start=(t == 0), stop=False)
                rhs_sc = x_pad[:, 2 * r0 + 3: 2 * r0 + 2 + 2 * nrow: 2,
                               1: 2 * W: 2]
                nc.tensor.matmul(ps[:, :nrow], lsc[:], rhs_sc,
                                 start=False, stop=True)
                nc.scalar.activation(fpadA[:, r0:r0 + nrow, 3:3 + W],
                                     ps[:, :nrow], AF.Relu,
                                     bias=beta2, scale=inv2)
            nc.vector.tensor_tensor(
                fpadA[:, 0:38], fpadA[:, 0:38],
                rmf[:, 0:38, None].to_broadcast((P, 38, FC)), AL.mult)

        # ================= Phase B: offsets -> idx + coeff maps =================
        # All per-(k,d) quantities live on partitions 0..17 with the quantity
        # index in the free dim (engines cannot cross partition bases).
        # Processed in 4 chunks of 1024 spatial positions (2 om blocks each).
        with tc.tile_pool(name="ph_b", bufs=1) as pb, \
             tc.tile_pool(name="ph_b_q", bufs=2) as pbq, \
             tc.tile_pool(name="ph_b_tmp", bufs=2) as pbt, \
             tc.tile_pool(name="psum_b", bufs=2, space="PSUM") as psb:
            fpadB = pb.tile([P, FR, FC], BF16)
            loff = pb.tile([P, 9, 54], BF16)
            yadd = pb.tile([18, S], F32)
            xadd = pb.tile([18, S], F32)
            idx16 = pb.tile([18, S], I16)
            nc.sync.dma_start(out=loff[:], in_=d_loff[:])
            nc.sync.dma_start(out=yadd[:], in_=d_yadd[:])
            nc.sync.dma_start(out=xadd[:], in_=d_xadd[:])
            nc.vector.tensor_copy(out=fpadB[:], in_=fpadA[:])

            for ch in range(4):
                # q_in rows 0..17, free: [quant, 1024]
                q_in = pbq.tile([18, 3, 1024], F32, tag="q_in")
                for cb2 in range(2):
                    cki = 2 * ch + cb2
                    # offset conv: out channels (quant*18 + 2k+d); i-order.
                    # block cki covers i in [512cki, 512cki+512):
                    # y = 2a + cki//4, x = 32*(cki%4) + j';
                    # rhs rows y+2+ty, cols x+2+tx, (j' outer, a inner)
                    b2, xq = cki // 4, 32 * (cki % 4)
                    ps = psb.tile([54, 512], F32)
                    for t in range(9):
                        ty, tx = t // 3, t % 3
                        rhs = fpadB[:, b2 + 2 + ty: b2 + 2 + ty + 32: 2,
                                    xq + 2 + tx: xq + 2 + tx + 32]
                        nc.tensor.matmul(
                            ps[:].rearrange("p (j a) -> p j a", j=32),
                            loff[:, t, :],
                            rhs.rearrange("p a j -> p j a"),
                            start=(t == 0), stop=(t == 8))
                    om_sb = pbt.tile([54, 512], F32, tag="om_sb")
                    nc.scalar.copy(om_sb[:], ps[:])
                    for q in range(3):
                        nc.gpsimd.dma_start(
                            out=q_in[:, q, cb2 * 512:(cb2 + 1) * 512],
                            in_=om_sb[q * 18:(q + 1) * 18, :])

                qd = pbt.tile([18, 7, 1024], F32, tag="qd")
                qb = pbt.tile([18, 5, 1024], BF16, tag="qb")
                qcc = pbt.tile([18, 4, 1024], BF16, tag="qcc")
                sl = slice(ch * 1024, (ch + 1) * 1024)
                dy, dx, mm = q_in[:, 0, :], q_in[:, 1, :], q_in[:, 2, :]
                y_, x_ = qd[:, 0, :], qd[:, 1, :]
                t1, t2 = qd[:, 2, :], qd[:, 3, :]
                y0, x0 = qd[:, 4, :], qd[:, 5, :]
                idxf = qd[:, 6, :]
                wy, wx, m_ = qb[:, 0, :], qb[:, 1, :], qb[:, 2, :]
                u_, t_ = qb[:, 3, :], qb[:, 4, :]

                nc.vector.scalar_tensor_tensor(
                    y_, dy, cst[0:18, 6:7], yadd[:, sl], AL.add, AL.add)
                nc.vector.scalar_tensor_tensor(
                    x_, dx, cst[0:18, 7:8], xadd[:, sl], AL.add, AL.add)
                # floor via RNE magic + correction
                nc.vector.tensor_scalar(t1, y_, MAGIC, -MAGIC, AL.add, AL.add)
                nc.vector.tensor_tensor(t2, y_, t1, AL.is_lt)
                nc.vector.tensor_tensor(y0, t1, t2, AL.subtract)
                nc.vector.tensor_scalar(t1, x_, MAGIC, -MAGIC, AL.add, AL.add)
                nc.vector.tensor_tensor(t2, x_, t1, AL.is_lt)
                nc.vector.tensor_tensor(x0, t1, t2, AL.subtract)
                nc.vector.tensor_tensor(wy, y_, y0, AL.subtract)
                nc.vector.tensor_tensor(wx, x_, x0, AL.subtract)
                nc.vector.scalar_tensor_tensor(idxf, y0, float(FC), x0,
                                               AL.mult, AL.add)
                nc.vector.tensor_scalar(idxf, idxf, float(IDXMAX), 0.0,
                                        AL.min, AL.max)
                nc.vector.tensor_copy(out=idx16[:, sl], in_=idxf)

                nc.scalar.activation(m_, mm, AF.Sigmoid, bias=cst[0:18, 8:9])
                nc.vector.tensor_tensor(u_, m_, wy, AL.mult)
                nc.vector.tensor_tensor(t_, m_, u_, AL.subtract)
                nc.vector.tensor_tensor(qcc[:, 3, :], u_, wx, AL.mult)
                nc.vector.tensor_tensor(qcc[:, 2, :], u_, qcc[:, 3, :],
                                        AL.subtract)
                nc.vector.tensor_tensor(qcc[:, 1, :], t_, wx, AL.mult)
                nc.vector.tensor_tensor(qcc[:, 0, :], t_, qcc[:, 1, :],
                                        AL.subtract)
                for j4 in range(4):
                    nc.sync.dma_start(out=d_cm[j4, :, sl],
                                      in_=qcc[:, j4, :])

            with nc.allow_non_contiguous_dma(reason="wrapped idx scatter"):
                for kd in range(18):
                    nc.sync.dma_start(
                        out=d_iw[kd].rearrange("p j -> j p"),
                        in_=idx16[kd:kd + 1, :])

        # ================= Phase C: gather + hadamard + einsum =================
        with tc.tile_pool(name="idxp", bufs=2) as idxp, \
             tc.tile_pool(name="cbp", bufs=3) as cbp, \
             tc.tile_pool(name="vp", bufs=1) as vp, \
             tc.tile_pool(name="pp", bufs=3) as ppool, \
             tc.tile_pool(name="psum_c", bufs=1, space="PSUM") as psc, \
             tc.tile_pool(name="outp", bufs=1) as outp:
            pos = psc.tile([P, S], F32)
            fflat = fpadA[:].rearrange("p a b -> p (a b)")
            for k in range(9):
                idxw = idxp.tile([P, 4, 256], I16, tag="idxw")
                for dd in range(2):
                    nc.sync.dma_start(
                        out=idxw[dd * 64:(dd + 1) * 64, 0, :],
                        in_=d_iw[2 * k + dd].rearrange("p j -> (p j)")[None, :]
                            .to_broadcast([4, S]))
                nc.vector.tensor_scalar_add(idxw[:, 1, :], idxw[:, 0, :], 1)
                nc.vector.tensor_scalar_add(idxw[:, 2, :], idxw[:, 0, :], FC)
                nc.vector.tensor_scalar_add(idxw[:, 3, :], idxw[:, 0, :], FC + 1)
                # two corners per gather: the cost model charges
                # max(out_free, num_elems) per instruction, so batch the
                # output well past the 5360-element source scan.
                for g2 in range(2):
                    v = vp.tile([P, 2 * S], F32, tag=f"v{g2}")
                    nc.gpsimd.ap_gather(
                        out_ap=v[:], in_ap=fflat,
                        idxs_ap=idxw[:, 2 * g2: 2 * g2 + 2, :]
                            .rearrange("p a b -> p (a b)"),
                        channels=P, num_elems=NELEM, d=1, num_idxs=2 * S)
                    for j2 in range(2):
                        j4 = 2 * g2 + j2
                        cb = cbp.tile([P, S], BF16, tag="cb")
                        nc.sync.dma_start(
                            out=cb[:],
                            in_=d_cm[j4, 2 * k: 2 * k + 2, None, :]
                                .to_broadcast([2, 64, S]))
                        pt = ppool.tile([P, S], BF16, tag="pt")
                        nc.vector.tensor_tensor(
                            pt[:], v[:, j2 * S:(j2 + 1) * S], cb[:], AL.mult)
                        for b4 in range(8):
                            nc.tensor.matmul(
                                pos[:, b4 * 512:(b4 + 1) * 512],
                                ldcn[:, k, :],
                                pt[:, b4 * 512:(b4 + 1) * 512],
                                start=(k == 0 and j4 == 0),
                                stop=(k == 8 and j4 == 3))

            # out stage: bn3 + relu, un-permute i-order -> (y, x)
            ob = outp.tile([P, S], F32)
            nc.scalar.activation(
                ob[:].rearrange("p (a b x) -> p b x a", a=16, b=2),
                pos[:].rearrange("p (b x a) -> p b x a", b=2, x=128),
                AF.Relu, bias=beta3, scale=inv3)
            nc.sync.dma_start(out=d_out[:],
                              in_=ob[:].rearrange("p (y x) -> p y x", y=QROWS))

    nc.compile()
    return nc


_CACHE = {}


def _prep(inputs):
    f = {k: _f(v) for k, v in inputs.items()}
    inv1 = f['g1'] / np.sqrt(f['v1'] + EPS)
    beta1 = f['b1'] - f['m1'] * inv1
    inv2 = f['g2'] / np.sqrt(f['v2'] + EPS)
    beta2 = f['b2'] - f['m2'] * inv2
    invd = f['gd'] / np.sqrt(f['vd'] + EPS)
    betad = f['bd'] - f['md'] * invd
    inv3 = f['g3'] / np.sqrt(f['v3'] + EPS)
    beta3 = f['b3'] - f['m3'] * inv3

    lhsT1 = np.transpose(f['w1'], (1, 2, 3, 0)).reshape(Ci, 9, P)
    lhsT2 = np.transpose(f['w2'], (1, 2, 3, 0)).reshape(P, 9, P)
    wd = f['wd'][:, :, 0, 0] * (invd / inv2)[:, None]
    lhsT_sc = np.ascontiguousarray(wd.T)

    # offset conv rows: quant*18 + k*2 + d  <-  orig quant*18 + d*9 + k
    perm = np.zeros(54, dtype=np.int64)
    for quant in range(3):
        for kk in range(9):
            for dd in range(2):
                perm[quant * 18 + kk * 2 + dd] = quant * 18 + dd * 9 + kk
    ow = f['off_w'][perm]
    obias = f['off_b'][perm]
    lhsT_off = np.transpose(ow, (1, 2, 3, 0)).reshape(P, 9, 54)

    wr = f['dcn_w'].reshape(Co, DG, Cg, 9)
    lhsT_dcn = np.transpose(wr, (1, 2, 3, 0)).reshape(P, 9, Co)

    cst = np.zeros((P, 10), dtype=np.float32)
    cst[:, 0], cst[:, 1] = inv1, beta1
    cst[:, 2], cst[:, 3] = inv2, beta2 + betad
    cst[:, 4], cst[:, 5] = inv3, beta3 + inv3 * f['dcn_b']
    for kd in range(18):
        cst[kd, 6] = obias[0 * 18 + kd]   # dy bias
        cst[kd, 7] = obias[1 * 18 + kd]   # dx bias
        cst[kd, 8] = obias[2 * 18 + kd]   # mask bias

    # i-order position constants: i = ((y%2)*128 + x)*16 + y//2
    ii = np.arange(S)
    aa = ii % 16
    cc = ii // 16
    bb2 = cc // 128
    xx = cc % 128
    yloc = 2 * aa + bb2
    y_add = np.zeros((18, S), dtype=np.float32)
    x_add = np.zeros((18, S), dtype=np.float32)
    for kk in range(9):
        for dd in range(2):
            kd = 2 * kk + dd
            y_add[kd] = yloc + (kk // 3) + 2
            x_add[kd] = xx + (kk % 3) + 2

    return dict(
        lhsT1=_f(lhsT1), lhsT2=_bf(lhsT2), lhsT_sc=_f(lhsT_sc),
        lhsT_off=_bf(lhsT_off), lhsT_dcn=_bf(lhsT_dcn),
        consts=_f(cst), y_add=_f(y_add), x_add=_f(x_add), x=f['x'])


def kernel(**inputs):
    cfg = _prep(inputs)
    x = cfg.pop('x')
    B = x.shape[0]

    if 'nc' not in _CACHE:
        _CACHE['nc'] = build_nc()
    nc = _CACHE['nc']

    in_maps = []
    for cid in range(8):
        b, q = cid // 4, cid % 4
        h0 = 32 * q
        xp = np.zeros((Ci, XR, XC), dtype=np.float32)
        r_lo = 2 * h0 - 9
        s_lo, s_hi = max(r_lo, 0), min(2 * h0 + 72, 256)
        xp[:, s_lo - r_lo: s_hi - r_lo, 1:257] = x[b, :, s_lo:s_hi, :]
        rm1 = np.zeros((P, F1R), dtype=np.float32)
        for f1 in range(F1R):
            rm1[:, f1] = 1.0 if 0 <= h0 - 4 + f1 < H else 0.0
        rmf = np.zeros((P, FR), dtype=np.float32)
        for f2 in range(38):
            rmf[:, f2] = 1.0 if 0 <= h0 - 3 + f2 < H else 0.0
        m = dict(cfg)
        m['x_shard'] = np.ascontiguousarray(xp)
        m['rowmask1'] = rm1
        m['rowmaskF'] = rmf
        in_maps.append(m)

    res = run_bass_kernel_spmd(nc, in_maps, core_ids=list(range(8)))
    out = np.zeros((B, Co, H, W), dtype=np.float32)
    for cid in range(8):
        b, q = cid // 4, cid % 4
        out[b, :, 32 * q:32 * q + 32, :] = res.results[cid]['out']
    return out


# revision 8
# speedup vs baseline: 3.2675x; 1.1960x over previous
"""Trainium2 Bass kernel for nn_DeforConv_71605694759687 (gather-based).

ResBlock(stride2, 64->128) + DCNv2 (modulated deformable conv) + BN + ReLU.

Sharding (8 cores): (batch b = core//4, H-quarter q = core%4); each core
computes 32 output rows of out[b] end-to-end locally (halo via recompute,
no collectives).

Unlike the tent-expansion design, deformable sampling here uses real
GPSIMD gathers (ap_gather): per 3x3 tap k, the four bilinear corner
values are gathered from the padded feature map at runtime-computed
int16 indices, multiplied by per-corner coefficient maps
(mask * bilinear weights, broadcast from 18 rows to 128 partitions via
DRAM-bounce replication DMAs), and contracted on the PE with the DCN
weights (9 taps x 4 corners accumulating matmuls).

Spatial positions use the "i-order" i = ((y%2)*128 + x)*16 + y//2 so
that the ap_gather 16-partition index wrap, the offset-conv rhs view,
and the output un-permute are all regular strided APs.
"""

import numpy as np
import ml_dtypes
from contextlib import ExitStack

import concourse.bass as bass
import concourse.tile as tile
from concourse import mybir, bacc
from concourse.bass_utils import run_bass_kernel_spmd

F32 = mybir.dt.float32
F32R = mybir.dt.float32r
BF16 = mybir.dt.bfloat16
I16 = mybir.dt.int16
AL = mybir.AluOpType
AF = mybir.ActivationFunctionType

P = 128
EPS = 1e-5
Ci, Co, DG, Cg = 64, 128, 2, 64
H, W = 128, 128          # output spatial (after stride-2)
QROWS = 32               # output rows per core
FR = 40                  # F_pad rows: h0-3 .. h0+34 (+2 zero guard rows)
FC = 134                 # F_pad cols: x in [-3, 130]
NELEM = FR * FC          # 5360 gather elements per partition
F1R, F1C = 40, 130       # feat1: rows h0-4..h0+35, cols [-1,128]
XR, XC = 81, 258         # x_pad: rows 2*h0-9..2*h0+71, cols [-1,256]
S = 4096                 # spatial positions per core (32*128)
MAGIC = 12582912.0       # 1.5 * 2^23, fp32 RNE rounding trick
IDXMAX = 37 * FC + 132   # max legal idx00


def _bf(x):
    return np.ascontiguousarray(np.asarray(x).astype(ml_dtypes.bfloat16))


def _f(x):
    return np.ascontiguousarray(np.asarray(x, dtype=np.float32))


def build_nc():
    nc = bacc.Bacc(None)

    d_x = nc.dram_tensor("x_shard", [Ci, XR, XC], F32R, kind="ExternalInput")
    d_l1 = nc.dram_tensor("lhsT1", [Ci, 9, P], F32R, kind="ExternalInput")
    d_l2 = nc.dram_tensor("lhsT2", [P, 9, P], BF16, kind="ExternalInput")
    d_lsc = nc.dram_tensor("lhsT_sc", [Ci, P], F32R, kind="ExternalInput")
    d_loff = nc.dram_tensor("lhsT_off", [P, 9, 54], BF16, kind="ExternalInput")
    d_ldcn = nc.dram_tensor("lhsT_dcn", [P, 9, P], BF16, kind="ExternalInput")
    d_cst = nc.dram_tensor("consts", [P, 12], F32, kind="ExternalInput")
    d_yadd = nc.dram_tensor("y_add", [18, S], F32, kind="ExternalInput")
    d_xadd = nc.dram_tensor("x_add", [18, S], F32, kind="ExternalInput")
    d_rm1 = nc.dram_tensor("rowmask1", [P, F1R], F32, kind="ExternalInput")
    d_rmf = nc.dram_tensor("rowmaskF", [P, FR], F32, kind="ExternalInput")
    d_out = nc.dram_tensor("out", [P, QROWS, W], F32, kind="ExternalOutput")

    d_cm = nc.dram_tensor("cmaps", [4, 18, S], BF16, kind="Internal")
    d_iw = nc.dram_tensor("idxw", [18, 16, 256], I16, kind="Internal")

    with tile.TileContext(nc) as tc, ExitStack() as ctx:
        singles = ctx.enter_context(tc.tile_pool(name="singles", bufs=1))

        fpadA = singles.tile([P, FR, FC], F32)     # gather source, col c <-> x-3
        ldcn = singles.tile([P, 9, P], BF16)
        cst = singles.tile([P, 12], F32)
        nc.sync.dma_start(out=ldcn[:], in_=d_ldcn[:])
        nc.sync.dma_start(out=cst[:], in_=d_cst[:])

        inv1, beta1 = cst[:, 0:1], cst[:, 1:2]
        inv2, beta2 = cst[:, 2:3], cst[:, 3:4]
        inv3, beta3 = cst[:, 4:5], cst[:, 5:6]

        nc.vector.memset(fpadA[:, :, 0:3], 0.0)
        nc.vector.memset(fpadA[:, :, FC - 3:FC], 0.0)
        nc.vector.memset(fpadA[:, 38:40, :], 0.0)

        # ================= Phase A: ResBlock =================
        with tc.tile_pool(name="ph_a", bufs=1) as pa, \
             tc.tile_pool(name="psum_a", bufs=2, space="PSUM") as psa:
            x_pad = pa.tile([Ci, XR, XC], F32R)
            feat1 = pa.tile([P, F1R, F1C], BF16)
            l1 = pa.tile([Ci, 9, P], F32R)
            l2 = pa.tile([P, 9, P], BF16)
            lsc = pa.tile([Ci, P], F32R)
            rm1 = pa.tile([P, F1R], F32)
            rmf = pa.tile([P, FR], F32)
            for i in range(8):
                r0, r1 = (XR * i) // 8, (XR * (i + 1)) // 8
                nc.sync.dma_start(out=x_pad[:, r0:r1, :],
                                  in_=d_x[:, r0:r1, :])
            for t, dref in ((l1, d_l1), (l2, d_l2),
                            (lsc, d_lsc), (rm1, d_rm1), (rmf, d_rmf)):
                nc.sync.dma_start(out=t[:], in_=dref[:])

            nc.vector.memset(feat1[:, :, 0:1], 0.0)
            nc.vector.memset(feat1[:, :, F1C - 1:F1C], 0.0)

            # conv1 3x3 s2 + bn1 + relu -> feat1 (bf16)
            for cki in range(10):
                r0 = cki * 4
                ps = psa.tile([P, 4, W], F32)
                for t in range(9):
                    ty, tx = t // 3, t % 3
                    rhs = x_pad[:, 2 * r0 + ty: 2 * r0 + ty + 7: 2,
                                tx: tx + 2 * W - 1: 2]
                    nc.tensor.matmul(ps[:], l1[:, t, :], rhs,
                                     start=(t == 0), stop=(t == 8))
                nc.scalar.activation(feat1[:, r0:r0 + 4, 1:1 + W], ps[:],
                                     AF.Relu, bias=beta1, scale=inv1)
            for ms in range(4):
                r0, r1 = ms * 10, (ms + 1) * 10
                nc.vector.tensor_tensor(
                    feat1[:, r0:r1], feat1[:, r0:r1],
                    rm1[:, r0:r1, None].to_broadcast((P, 10, F1C)), AL.mult)

            # conv2 3x3 s1 (+ folded shortcut) + bn + relu -> fpadA rows 0..37
            for cki in range(10):
                r0 = cki * 4
                nrow = min(4, 38 - r0)
                ps = psa.tile([P, 4, W], F32, tag="ps2")
                for t in range(9):
                    ty, tx = t // 3, t % 3
                    rhs = feat1[:, r0 + ty: r0 + ty + nrow, tx: tx + W]
                    nc.tensor.matmul(ps[:, :nrow], l2[:, t, :], rhs,
                                     start=(t == 0), stop=False)
                rhs_sc = x_pad[:, 2 * r0 + 3: 2 * r0 + 2 + 2 * nrow: 2,
                               1: 2 * W: 2]
                nc.tensor.matmul(ps[:, :nrow], lsc[:], rhs_sc,
                                 start=False, stop=True)
                nc.scalar.activation(fpadA[:, r0:r0 + nrow, 3:3 + W],
                                     ps[:, :nrow], AF.Relu,
                                     bias=beta2, scale=inv2)
            nc.vector.tensor_tensor(
                fpadA[:, 0:38], fpadA[:, 0:38],
                rmf[:, 0:38, None].to_broadcast((P, 38, FC)), AL.mult)

        # ================= Phase B: offsets -> idx + coeff maps =================
        # All per-(k,d) quantities live on partitions 0..17 with the quantity
        # index in the free dim (engines cannot cross partition bases).
        # Processed in 4 chunks of 1024 spatial positions (2 om blocks each).
        with tc.tile_pool(name="ph_b", bufs=1) as pb, \
             tc.tile_pool(name="ph_b_q", bufs=2) as pbq, \
             tc.tile_pool(name="ph_b_tmp", bufs=2) as pbt, \
             tc.tile_pool(name="psum_b", bufs=2, space="PSUM") as psb:
            fpadB = pb.tile([P, FR, FC], BF16)
            loff = pb.tile([P, 9, 54], BF16)
            yadd = pb.tile([18, S], F32)
            xadd = pb.tile([18, S], F32)
            idx16 = pb.tile([18, S], I16)
            nc.sync.dma_start(out=loff[:], in_=d_loff[:])
            nc.sync.dma_start(out=yadd[:], in_=d_yadd[:])
            nc.sync.dma_start(out=xadd[:], in_=d_xadd[:])
            nc.vector.tensor_copy(out=fpadB[:], in_=fpadA[:])

            for ch in range(4):
                # q_in rows 0..17, free: [quant, 1024]
                q_in = pbq.tile([18, 3, 1024], F32, tag="q_in")
                for cb2 in range(2):
                    cki = 2 * ch + cb2
                    # offset conv: out channels (quant*18 + 2k+d); i-order.
                    # block cki covers i in [512cki, 512cki+512):
                    # y = 2a + cki//4, x = 32*(cki%4) + j';
                    # rhs rows y+2+ty, cols x+2+tx, (j' outer, a inner)
                    b2, xq = cki // 4, 32 * (cki % 4)
                    ps = psb.tile([54, 512], F32)
                    for t in range(9):
                        ty, tx = t // 3, t % 3
                        rhs = fpadB[:, b2 + 2 + ty: b2 + 2 + ty + 32: 2,
                                    xq + 2 + tx: xq + 2 + tx + 32]
                        nc.tensor.matmul(
                            ps[:].rearrange("p (j a) -> p j a", j=32),
                            loff[:, t, :],
                            rhs.rearrange("p a j -> p j a"),
                            start=(t == 0), stop=(t == 8))
                    om_sb = pbt.tile([54, 512], F32, tag="om_sb")
                    nc.scalar.copy(om_sb[:], ps[:])
                    for q in range(3):
                        nc.gpsimd.dma_start(
                            out=q_in[:, q, cb2 * 512:(cb2 + 1) * 512],
                            in_=om_sb[q * 18:(q + 1) * 18, :])

                qd = pbt.tile([18, 7, 1024], F32, tag="qd")
                qb = pbt.tile([18, 5, 1024], BF16, tag="qb")
                qcc = pbt.tile([18, 4, 1024], BF16, tag="qcc")
                sl = slice(ch * 1024, (ch + 1) * 1024)
                dy, dx, mm = q_in[:, 0, :], q_in[:, 1, :], q_in[:, 2, :]
                y_, x_ = qd[:, 0, :], qd[:, 1, :]
                t1, t2 = qd[:, 2, :], qd[:, 3, :]
                y0, x0 = qd[:, 4, :], qd[:, 5, :]
                idxf = qd[:, 6, :]
                wy, wx, m_ = qb[:, 0, :], qb[:, 1, :], qb[:, 2, :]
                u_, t_ = qb[:, 3, :], qb[:, 4, :]

                nc.vector.scalar_tensor_tensor(
                    y_, dy, cst[0:18, 6:7], yadd[:, sl], AL.add, AL.add)
                nc.vector.scalar_tensor_tensor(
                    x_, dx, cst[0:18, 7:8], xadd[:, sl], AL.add, AL.add)
                # y' = y - 0.5 (folded into yadd); y0 = RNE(y') = floor(y)
                # except at exact-integer y, where wy=1 keeps bilinear exact.
                nc.scalar.activation(t1, y_, AF.Identity, bias=cst[0:18, 9:10])
                nc.scalar.activation(y0, t1, AF.Identity, bias=cst[0:18, 10:11])
                nc.scalar.activation(t2, x_, AF.Identity, bias=cst[0:18, 9:10])
                nc.scalar.activation(x0, t2, AF.Identity, bias=cst[0:18, 10:11])
                nc.vector.scalar_tensor_tensor(wy, y_, 0.5, y0,
                                               AL.add, AL.subtract)
                nc.vector.scalar_tensor_tensor(wx, x_, 0.5, x0,
                                               AL.add, AL.subtract)
                nc.vector.scalar_tensor_tensor(idxf, y0, float(FC), x0,
                                               AL.mult, AL.add)
                nc.vector.tensor_scalar(idxf, idxf, float(IDXMAX), 0.0,
                                        AL.min, AL.max)
                nc.scalar.copy(out=idx16[:, sl], in_=idxf)

                nc.scalar.activation(m_, mm, AF.Sigmoid, bias=cst[0:18, 8:9])
                nc.vector.tensor_tensor(u_, m_, wy, AL.mult)
                nc.vector.tensor_tensor(t_, m_, u_, AL.subtract)
                nc.vector.tensor_tensor(qcc[:, 3, :], u_, wx, AL.mult)
                nc.vector.tensor_tensor(qcc[:, 2, :], u_, qcc[:, 3, :],
                                        AL.subtract)
                nc.vector.tensor_tensor(qcc[:, 1, :], t_, wx, AL.mult)
                nc.vector.tensor_tensor(qcc[:, 0, :], t_, qcc[:, 1, :],
                                        AL.subtract)
                for j4 in range(4):
                    nc.sync.dma_start(out=d_cm[j4, :, sl],
                                      in_=qcc[:, j4, :])
                with nc.allow_non_contiguous_dma(reason="wrapped idx scatter"):
                    for kd in range(18):
                        nc.sync.dma_start(
                            out=d_iw[kd][:, ch * 64:(ch + 1) * 64]
                                .rearrange("p j -> j p"),
                            in_=idx16[kd:kd + 1, sl])

        # ================= Phase C: gather + hadamard + einsum =================
        with tc.tile_pool(name="idxp", bufs=1) as idxp, \
             tc.tile_pool(name="cbp", bufs=3) as cbp, \
             tc.tile_pool(name="vp", bufs=1) as vp, \
             tc.tile_pool(name="pp", bufs=3) as ppool, \
             tc.tile_pool(name="psum_c", bufs=1, space="PSUM") as psc, \
             tc.tile_pool(name="outp", bufs=1) as outp:
            pos = psc.tile([P, S], F32)
            fflat = fpadA[:].rearrange("p a b -> p (a b)")
            idxall = idxp.tile([P, 9, 4, 256], I16)
            for k in range(9):
                for dd in range(2):
                    nc.sync.dma_start(
                        out=idxall[dd * 64:(dd + 1) * 64, k, 0, :],
                        in_=d_iw[2 * k + dd].rearrange("p j -> (p j)")[None, :]
                            .to_broadcast([4, S]))
                nc.vector.tensor_scalar_add(idxall[:, k, 1, :],
                                            idxall[:, k, 0, :], 1)
                nc.vector.tensor_scalar_add(idxall[:, k, 2, :],
                                            idxall[:, k, 0, :], FC)
                nc.vector.tensor_scalar_add(idxall[:, k, 3, :],
                                            idxall[:, k, 0, :], FC + 1)
            for k in range(9):
                idxw = idxall[:, k]
                # two corners per gather: the cost model charges
                # max(out_free, num_elems) per instruction, so batch the
                # output well past the 5360-element source scan.
                for g2 in range(2):
                    v = vp.tile([P, 2 * S], F32, tag=f"v{g2}")
                    nc.gpsimd.ap_gather(
                        out_ap=v[:], in_ap=fflat,
                        idxs_ap=idxw[:, 2 * g2: 2 * g2 + 2, :]
                            .rearrange("p a b -> p (a b)"),
                        channels=P, num_elems=NELEM, d=1, num_idxs=2 * S)
                    for j2 in range(2):
                        j4 = 2 * g2 + j2
                        cb = cbp.tile([P, S], BF16, tag="cb")
                        nc.sync.dma_start(
                            out=cb[:],
                            in_=d_cm[j4, 2 * k: 2 * k + 2, None, :]
                                .to_broadcast([2, 64, S]))
                        pt = ppool.tile([P, S], BF16, tag="pt")
                        nc.vector.tensor_tensor(
                            pt[:], v[:, j2 * S:(j2 + 1) * S], cb[:], AL.mult)
                        for b4 in range(8):
                            nc.tensor.matmul(
                                pos[:, b4 * 512:(b4 + 1) * 512],
                                ldcn[:, k, :],
                                pt[:, b4 * 512:(b4 + 1) * 512],
                                start=(k == 0 and j4 == 0),
                                stop=(k == 8 and j4 == 3))

            # out stage: bn3 + relu, un-permute i-order -> (y, x)
            ob = outp.tile([P, S], F32)
            nc.scalar.activation(
                ob[:].rearrange("p (a b x) -> p b x a", a=16, b=2),
                pos[:].rearrange("p (b x a) -> p b x a", b=2, x=128),
                AF.Relu, bias=beta3, scale=inv3)
            nc.sync.dma_start(out=d_out[:],
                              in_=ob[:].rearrange("p (y x) -> p y x", y=QROWS))

    nc.compile()
    return nc


_CACHE = {}


def _prep(inputs):
    f = {k: _f(v) for k, v in inputs.items()}
    inv1 = f['g1'] / np.sqrt(f['v1'] + EPS)
    beta1 = f['b1'] - f['m1'] * inv1
    inv2 = f['g2'] / np.sqrt(f['v2'] + EPS)
    beta2 = f['b2'] - f['m2'] * inv2
    invd = f['gd'] / np.sqrt(f['vd'] + EPS)
    betad = f['bd'] - f['md'] * invd
    inv3 = f['g3'] / np.sqrt(f['v3'] + EPS)
    beta3 = f['b3'] - f['m3'] * inv3

    lhsT1 = np.transpose(f['w1'], (1, 2, 3, 0)).reshape(Ci, 9, P)
    lhsT2 = np.transpose(f['w2'], (1, 2, 3, 0)).reshape(P, 9, P)
    wd = f['wd'][:, :, 0, 0] * (invd / inv2)[:, None]
    lhsT_sc = np.ascontiguousarray(wd.T)

    # offset conv rows: quant*18 + k*2 + d  <-  orig quant*18 + d*9 + k
    perm = np.zeros(54, dtype=np.int64)
    for quant in range(3):
        for kk in range(9):
            for dd in range(2):
                perm[quant * 18 + kk * 2 + dd] = quant * 18 + dd * 9 + kk
    ow = f['off_w'][perm]
    obias = f['off_b'][perm]
    lhsT_off = np.transpose(ow, (1, 2, 3, 0)).reshape(P, 9, 54)

    wr = f['dcn_w'].reshape(Co, DG, Cg, 9)
    lhsT_dcn = np.transpose(wr, (1, 2, 3, 0)).reshape(P, 9, Co)

    cst = np.zeros((P, 12), dtype=np.float32)
    cst[:, 9], cst[:, 10] = MAGIC, -MAGIC
    cst[:, 0], cst[:, 1] = inv1, beta1
    cst[:, 2], cst[:, 3] = inv2, beta2 + betad
    cst[:, 4], cst[:, 5] = inv3, beta3 + inv3 * f['dcn_b']
    for kd in range(18):
        cst[kd, 6] = obias[0 * 18 + kd]   # dy bias
        cst[kd, 7] = obias[1 * 18 + kd]   # dx bias
        cst[kd, 8] = obias[2 * 18 + kd]   # mask bias

    # i-order position constants: i = ((y%2)*128 + x)*16 + y//2
    ii = np.arange(S)
    aa = ii % 16
    cc = ii // 16
    bb2 = cc // 128
    xx = cc % 128
    yloc = 2 * aa + bb2
    y_add = np.zeros((18, S), dtype=np.float32)
    x_add = np.zeros((18, S), dtype=np.float32)
    for kk in range(9):
        for dd in range(2):
            kd = 2 * kk + dd
            y_add[kd] = yloc + (kk // 3) + 1.5
            x_add[kd] = xx + (kk % 3) + 1.5

    return dict(
        lhsT1=_f(lhsT1), lhsT2=_bf(lhsT2), lhsT_sc=_f(lhsT_sc),
        lhsT_off=_bf(lhsT_off), lhsT_dcn=_bf(lhsT_dcn),
        consts=_f(cst), y_add=_f(y_add), x_add=_f(x_add), x=f['x'])


def kernel(**inputs):
    cfg = _prep(inputs)
    x = cfg.pop('x')
    B = x.shape[0]

    if 'nc' not in _CACHE:
        _CACHE['nc'] = build_nc()
    nc = _CACHE['nc']

    in_maps = []
    for cid in range(8):
        b, q = cid // 4, cid % 4
        h0 = 32 * q
        xp = np.zeros((Ci, XR, XC), dtype=np.float32)
        r_lo = 2 * h0 - 9
        s_lo, s_hi = max(r_lo, 0), min(2 * h0 + 72, 256)
        xp[:, s_lo - r_lo: s_hi - r_lo, 1:257] = x[b, :, s_lo:s_hi, :]
        rm1 = np.zeros((P, F1R), dtype=np.float32)
        for f1 in range(F1R):
            rm1[:, f1] = 1.0 if 0 <= h0 - 4 + f1 < H else 0.0
        rmf = np.zeros((P, FR), dtype=np.float32)
        for f2 in range(38):
            rmf[:, f2] = 1.0 if 0 <= h0 - 3 + f2 < H else 0.0
        m = dict(cfg)
        m['x_shard'] = np.ascontiguousarray(xp)
        m['rowmask1'] = rm1
        m['rowmaskF'] = rmf
        in_maps.append(m)

    res = run_bass_kernel_spmd(nc, in_maps, core_ids=list(range(8)))
    out = np.zeros((B, Co, H, W), dtype=np.float32)
    for cid in range(8):
        b, q = cid // 4, cid % 4
        out[b, :, 32 * q:32 * q + 32, :] = res.results[cid]['out']
    return out


# revision 9
# speedup vs baseline: 3.4770x; 1.0641x over previous
"""Trainium2 Bass kernel for nn_DeforConv_71605694759687 (gather-based).

ResBlock(stride2, 64->128) + DCNv2 (modulated deformable conv) + BN + ReLU.

Sharding (8 cores): (batch b = core//4, H-quarter q = core%4); each core
computes 32 output rows of out[b] end-to-end locally (halo via recompute,
no collectives).

Unlike the tent-expansion design, deformable sampling here uses real
GPSIMD gathers (ap_gather): per 3x3 tap k, the four bilinear corner
values are gathered from the padded feature map at runtime-computed
int16 indices, multiplied by per-corner coefficient maps
(mask * bilinear weights, broadcast from 18 rows to 128 partitions via
DRAM-bounce replication DMAs), and contracted on the PE with the DCN
weights (9 taps x 4 corners accumulating matmuls).

Spatial positions use the "i-order" i = ((y%2)*128 + x)*16 + y//2 so
that the ap_gather 16-partition index wrap, the offset-conv rhs view,
and the output un-permute are all regular strided APs.
"""

import numpy as np
import ml_dtypes
from contextlib import ExitStack

import concourse.bass as bass
import concourse.tile as tile
from concourse import mybir, bacc
from concourse.bass_utils import run_bass_kernel_spmd

F32 = mybir.dt.float32
F32R = mybir.dt.float32r
BF16 = mybir.dt.bfloat16
I16 = mybir.dt.int16
AL = mybir.AluOpType
AF = mybir.ActivationFunctionType

P = 128
EPS = 1e-5
Ci, Co, DG, Cg = 64, 128, 2, 64
H, W = 128, 128          # output spatial (after stride-2)
QROWS = 32               # output rows per core
FR = 40                  # F_pad rows: h0-3 .. h0+34 (+2 zero guard rows)
FC = 134                 # F_pad cols: x in [-3, 130]
NELEM = FR * FC          # 5360 gather elements per partition
F1R, F1C = 40, 130       # feat1: rows h0-4..h0+35, cols [-1,128]
XR, XC = 81, 258         # x_pad: rows 2*h0-9..2*h0+71, cols [-1,256]
S = 4096                 # spatial positions per core (32*128)
MAGIC = 12582912.0       # 1.5 * 2^23, fp32 RNE rounding trick
IDXMAX = 37 * FC + 132   # max legal idx00


def _bf(x):
    return np.ascontiguousarray(np.asarray(x).astype(ml_dtypes.bfloat16))


def _f(x):
    return np.ascontiguousarray(np.asarray(x, dtype=np.float32))


def build_nc():
    nc = bacc.Bacc(None)

    d_x = nc.dram_tensor("x_shard", [Ci, XR, XC], F32R, kind="ExternalInput")
    d_l1 = nc.dram_tensor("lhsT1", [Ci, 9, P], F32R, kind="ExternalInput")
    d_l2 = nc.dram_tensor("lhsT2", [P, 9, P], BF16, kind="ExternalInput")
    d_lsc = nc.dram_tensor("lhsT_sc", [Ci, P], F32R, kind="ExternalInput")
    d_loff = nc.dram_tensor("lhsT_off", [P, 9, 54], BF16, kind="ExternalInput")
    d_ldcn = nc.dram_tensor("lhsT_dcn", [P, 9, P], BF16, kind="ExternalInput")
    d_cst = nc.dram_tensor("consts", [P, 12], F32, kind="ExternalInput")
    d_yadd = nc.dram_tensor("y_add", [18, S], F32, kind="ExternalInput")
    d_xadd = nc.dram_tensor("x_add", [18, S], F32, kind="ExternalInput")
    d_rm1 = nc.dram_tensor("rowmask1", [P, F1R], F32, kind="ExternalInput")
    d_rmf = nc.dram_tensor("rowmaskF", [P, FR], F32, kind="ExternalInput")
    d_out = nc.dram_tensor("out", [P, QROWS, W], F32, kind="ExternalOutput")

    d_cm = nc.dram_tensor("cmaps", [4, 18, S], BF16, kind="Internal")
    d_iw = nc.dram_tensor("idxw", [18, 16, 256], I16, kind="Internal")

    with tile.TileContext(nc) as tc, ExitStack() as ctx:
        singles = ctx.enter_context(tc.tile_pool(name="singles", bufs=1))

        fpadA = singles.tile([P, FR, FC], F32)     # gather source, col c <-> x-3
        ldcn = singles.tile([P, 9, P], BF16)
        cst = singles.tile([P, 12], F32)
        nc.sync.dma_start(out=ldcn[:], in_=d_ldcn[:])
        nc.sync.dma_start(out=cst[:], in_=d_cst[:])

        inv1, beta1 = cst[:, 0:1], cst[:, 1:2]
        inv2, beta2 = cst[:, 2:3], cst[:, 3:4]
        inv3, beta3 = cst[:, 4:5], cst[:, 5:6]

        nc.vector.memset(fpadA[:, :, 0:3], 0.0)
        nc.vector.memset(fpadA[:, :, FC - 3:FC], 0.0)
        nc.vector.memset(fpadA[:, 38:40, :], 0.0)

        # ================= Phase A: ResBlock =================
        with tc.tile_pool(name="ph_a", bufs=1) as pa, \
             tc.tile_pool(name="psum_a", bufs=2, space="PSUM") as psa:
            x_pad = pa.tile([Ci, XR, XC], F32R)
            feat1 = pa.tile([P, F1R, F1C], BF16)
            l1 = pa.tile([Ci, 9, P], F32R)
            l2 = pa.tile([P, 9, P], BF16)
            lsc = pa.tile([Ci, P], F32R)
            rm1 = pa.tile([P, F1R], F32)
            rmf = pa.tile([P, FR], F32)
            for i in range(8):
                r0, r1 = (XR * i) // 8, (XR * (i + 1)) // 8
                nc.sync.dma_start(out=x_pad[:, r0:r1, :],
                                  in_=d_x[:, r0:r1, :])
            for t, dref in ((l1, d_l1), (l2, d_l2),
                            (lsc, d_lsc), (rm1, d_rm1), (rmf, d_rmf)):
                nc.sync.dma_start(out=t[:], in_=dref[:])

            nc.vector.memset(feat1[:, :, 0:1], 0.0)
            nc.vector.memset(feat1[:, :, F1C - 1:F1C], 0.0)

            # conv1 3x3 s2 + bn1 + relu -> feat1 (bf16)
            for cki in range(10):
                r0 = cki * 4
                ps = psa.tile([P, 4, W], F32)
                for t in range(9):
                    ty, tx = t // 3, t % 3
                    rhs = x_pad[:, 2 * r0 + ty: 2 * r0 + ty + 7: 2,
                                tx: tx + 2 * W - 1: 2]
                    nc.tensor.matmul(ps[:], l1[:, t, :], rhs,
                                     start=(t == 0), stop=(t == 8))
                nc.scalar.activation(feat1[:, r0:r0 + 4, 1:1 + W], ps[:],
                                     AF.Relu, bias=beta1, scale=inv1)
            for ms in range(4):
                r0, r1 = ms * 10, (ms + 1) * 10
                nc.vector.tensor_tensor(
                    feat1[:, r0:r1], feat1[:, r0:r1],
                    rm1[:, r0:r1, None].to_broadcast((P, 10, F1C)), AL.mult)

            # conv2 3x3 s1 (+ folded shortcut) + bn + relu -> fpadA rows 0..37
            for cki in range(10):
                r0 = cki * 4
                nrow = min(4, 38 - r0)
                ps = psa.tile([P, 4, W], F32, tag="ps2")
                for t in range(9):
                    ty, tx = t // 3, t % 3
                    rhs = feat1[:, r0 + ty: r0 + ty + nrow, tx: tx + W]
                    nc.tensor.matmul(ps[:, :nrow], l2[:, t, :], rhs,
                                     start=(t == 0), stop=False)
                rhs_sc = x_pad[:, 2 * r0 + 3: 2 * r0 + 2 + 2 * nrow: 2,
                               1: 2 * W: 2]
                nc.tensor.matmul(ps[:, :nrow], lsc[:], rhs_sc,
                                 start=False, stop=True)
                nc.scalar.activation(fpadA[:, r0:r0 + nrow, 3:3 + W],
                                     ps[:, :nrow], AF.Relu,
                                     bias=beta2, scale=inv2)
            nc.vector.tensor_tensor(
                fpadA[:, 0:38], fpadA[:, 0:38],
                rmf[:, 0:38, None].to_broadcast((P, 38, FC)), AL.mult)

        # ================= Phase B: offsets -> idx + coeff maps =================
        # All per-(k,d) quantities live on partitions 0..17 with the quantity
        # index in the free dim (engines cannot cross partition bases).
        # Processed in 4 chunks of 1024 spatial positions (2 om blocks each).
        with tc.tile_pool(name="ph_b", bufs=1) as pb, \
             tc.tile_pool(name="ph_b_q", bufs=2) as pbq, \
             tc.tile_pool(name="ph_b_tmp", bufs=2) as pbt, \
             tc.tile_pool(name="psum_b", bufs=2, space="PSUM") as psb:
            fpadB = pb.tile([P, FR, FC], BF16)
            loff = pb.tile([P, 9, 54], BF16)
            yadd = pb.tile([18, S], F32)
            xadd = pb.tile([18, S], F32)
            idx16 = pb.tile([18, 16, 256], I16)   # wrapped (p, j) layout
            nc.sync.dma_start(out=loff[:], in_=d_loff[:])
            nc.sync.dma_start(out=yadd[:], in_=d_yadd[:])
            nc.sync.dma_start(out=xadd[:], in_=d_xadd[:])
            nc.vector.tensor_copy(out=fpadB[:], in_=fpadA[:])

            for ch in range(4):
                # q_in rows 0..17, free: [quant, 1024]
                q_in = pbq.tile([18, 3, 1024], F32, tag="q_in")
                for cb2 in range(2):
                    cki = 2 * ch + cb2
                    # offset conv: out channels (quant*18 + 2k+d); i-order.
                    # block cki covers i in [512cki, 512cki+512):
                    # y = 2a + cki//4, x = 32*(cki%4) + j';
                    # rhs rows y+2+ty, cols x+2+tx, (j' outer, a inner)
                    b2, xq = cki // 4, 32 * (cki % 4)
                    ps = psb.tile([54, 512], F32)
                    for t in range(9):
                        ty, tx = t // 3, t % 3
                        rhs = fpadB[:, b2 + 2 + ty: b2 + 2 + ty + 32: 2,
                                    xq + 2 + tx: xq + 2 + tx + 32]
                        nc.tensor.matmul(
                            ps[:].rearrange("p (j a) -> p j a", j=32),
                            loff[:, t, :],
                            rhs.rearrange("p a j -> p j a"),
                            start=(t == 0), stop=(t == 8))
                    om_sb = pbt.tile([54, 512], F32, tag="om_sb")
                    nc.scalar.copy(om_sb[:], ps[:])
                    for q in range(3):
                        nc.gpsimd.dma_start(
                            out=q_in[:, q, cb2 * 512:(cb2 + 1) * 512],
                            in_=om_sb[q * 18:(q + 1) * 18, :])

                qd = pbt.tile([18, 7, 1024], F32, tag="qd")
                qb = pbt.tile([18, 5, 1024], BF16, tag="qb")
                qcc = pbt.tile([18, 4, 1024], BF16, tag="qcc")
                sl = slice(ch * 1024, (ch + 1) * 1024)
                dy, dx, mm = q_in[:, 0, :], q_in[:, 1, :], q_in[:, 2, :]
                y_, x_ = qd[:, 0, :], qd[:, 1, :]
                t1, t2 = qd[:, 2, :], qd[:, 3, :]
                y0, x0 = qd[:, 4, :], qd[:, 5, :]
                idxf = qd[:, 6, :]
                wy, wx, m_ = qb[:, 0, :], qb[:, 1, :], qb[:, 2, :]
                u_, t_ = qb[:, 3, :], qb[:, 4, :]

                nc.vector.scalar_tensor_tensor(
                    y_, dy, cst[0:18, 6:7], yadd[:, sl], AL.add, AL.add)
                nc.vector.scalar_tensor_tensor(
                    x_, dx, cst[0:18, 7:8], xadd[:, sl], AL.add, AL.add)
                # y' = y - 0.5 (folded into yadd); y0 = RNE(y') = floor(y)
                # except at exact-integer y, where wy=1 keeps bilinear exact.
                nc.scalar.activation(t1, y_, AF.Identity, bias=cst[0:18, 9:10])
                nc.scalar.activation(y0, t1, AF.Identity, bias=cst[0:18, 10:11])
                nc.scalar.activation(t2, x_, AF.Identity, bias=cst[0:18, 9:10])
                nc.scalar.activation(x0, t2, AF.Identity, bias=cst[0:18, 10:11])
                nc.vector.scalar_tensor_tensor(wy, y_, 0.5, y0,
                                               AL.add, AL.subtract)
                nc.vector.scalar_tensor_tensor(wx, x_, 0.5, x0,
                                               AL.add, AL.subtract)
                nc.vector.scalar_tensor_tensor(idxf, y0, float(FC), x0,
                                               AL.mult, AL.add)
                nc.vector.tensor_scalar(idxf, idxf, float(IDXMAX), 0.0,
                                        AL.min, AL.max)
                nc.scalar.copy(
                    out=idx16[:, :, ch * 64:(ch + 1) * 64]
                        .rearrange("p q j -> p j q"),
                    in_=idxf)

                nc.scalar.activation(m_, mm, AF.Sigmoid, bias=cst[0:18, 8:9])
                nc.vector.tensor_tensor(u_, m_, wy, AL.mult)
                nc.vector.tensor_tensor(t_, m_, u_, AL.subtract)
                nc.vector.tensor_tensor(qcc[:, 3, :], u_, wx, AL.mult)
                nc.vector.tensor_tensor(qcc[:, 2, :], u_, qcc[:, 3, :],
                                        AL.subtract)
                nc.vector.tensor_tensor(qcc[:, 1, :], t_, wx, AL.mult)
                nc.vector.tensor_tensor(qcc[:, 0, :], t_, qcc[:, 1, :],
                                        AL.subtract)
                for j4 in range(4):
                    nc.sync.dma_start(out=d_cm[j4, :, sl],
                                      in_=qcc[:, j4, :])
                nc.sync.dma_start(
                    out=d_iw[:, :, ch * 64:(ch + 1) * 64],
                    in_=idx16[:, :, ch * 64:(ch + 1) * 64])

        # ================= Phase C: gather + hadamard + einsum =================
        with tc.tile_pool(name="idxp", bufs=1) as idxp, \
             tc.tile_pool(name="cbp", bufs=3) as cbp, \
             tc.tile_pool(name="vp", bufs=1) as vp, \
             tc.tile_pool(name="pp", bufs=3) as ppool, \
             tc.tile_pool(name="psum_c", bufs=1, space="PSUM") as psc, \
             tc.tile_pool(name="outp", bufs=1) as outp:
            pos = psc.tile([P, S], F32)
            fflat = fpadA[:].rearrange("p a b -> p (a b)")
            idxall = idxp.tile([P, 9, 4, 256], I16)
            for k in range(9):
                for dd in range(2):
                    nc.sync.dma_start(
                        out=idxall[dd * 64:(dd + 1) * 64, k, 0, :],
                        in_=d_iw[2 * k + dd].rearrange("p j -> (p j)")[None, :]
                            .to_broadcast([4, S]))
                nc.vector.tensor_scalar_add(idxall[:, k, 1, :],
                                            idxall[:, k, 0, :], 1)
                nc.vector.tensor_scalar_add(idxall[:, k, 2, :],
                                            idxall[:, k, 0, :], FC)
                nc.vector.tensor_scalar_add(idxall[:, k, 3, :],
                                            idxall[:, k, 0, :], FC + 1)
            for k in range(9):
                idxw = idxall[:, k]
                # two corners per gather: the cost model charges
                # max(out_free, num_elems) per instruction, so batch the
                # output well past the 5360-element source scan.
                for g2 in range(2):
                    v = vp.tile([P, 2 * S], F32, tag=f"v{g2}")
                    nc.gpsimd.ap_gather(
                        out_ap=v[:], in_ap=fflat,
                        idxs_ap=idxw[:, 2 * g2: 2 * g2 + 2, :]
                            .rearrange("p a b -> p (a b)"),
                        channels=P, num_elems=NELEM, d=1, num_idxs=2 * S)
                    for j2 in range(2):
                        j4 = 2 * g2 + j2
                        cb = cbp.tile([P, S], BF16, tag="cb")
                        nc.sync.dma_start(
                            out=cb[:],
                            in_=d_cm[j4, 2 * k: 2 * k + 2, None, :]
                                .to_broadcast([2, 64, S]))
                        pt = ppool.tile([P, S], BF16, tag="pt")
                        nc.vector.tensor_tensor(
                            pt[:], v[:, j2 * S:(j2 + 1) * S], cb[:], AL.mult)
                        for b4 in range(8):
                            nc.tensor.matmul(
                                pos[:, b4 * 512:(b4 + 1) * 512],
                                ldcn[:, k, :],
                                pt[:, b4 * 512:(b4 + 1) * 512],
                                start=(k == 0 and j4 == 0),
                                stop=(k == 8 and j4 == 3))

            # out stage: bn3 + relu, un-permute i-order -> (y, x)
            ob = outp.tile([P, S], F32)
            obp = ob[:].rearrange("p (a b x) -> p b x a", a=16, b=2)
            posp = pos[:].rearrange("p (b x a) -> p b x a", b=2, x=128)
            for oh in range(2):
                nc.scalar.activation(obp[:, oh], posp[:, oh],
                                     AF.Relu, bias=beta3, scale=inv3)
            nc.sync.dma_start(out=d_out[:],
                              in_=ob[:].rearrange("p (y x) -> p y x", y=QROWS))

    nc.compile()
    return nc


_CACHE = {}


def _prep(inputs):
    f = {k: _f(v) for k, v in inputs.items()}
    inv1 = f['g1'] / np.sqrt(f['v1'] + EPS)
    beta1 = f['b1'] - f['m1'] * inv1
    inv2 = f['g2'] / np.sqrt(f['v2'] + EPS)
    beta2 = f['b2'] - f['m2'] * inv2
    invd = f['gd'] / np.sqrt(f['vd'] + EPS)
    betad = f['bd'] - f['md'] * invd
    inv3 = f['g3'] / np.sqrt(f['v3'] + EPS)
    beta3 = f['b3'] - f['m3'] * inv3

    lhsT1 = np.transpose(f['w1'], (1, 2, 3, 0)).reshape(Ci, 9, P)
    lhsT2 = np.transpose(f['w2'], (1, 2, 3, 0)).reshape(P, 9, P)
    wd = f['wd'][:, :, 0, 0] * (invd / inv2)[:, None]
    lhsT_sc = np.ascontiguousarray(wd.T)

    # offset conv rows: quant*18 + k*2 + d  <-  orig quant*18 + d*9 + k
    perm = np.zeros(54, dtype=np.int64)
    for quant in range(3):
        for kk in range(9):
            for dd in range(2):
                perm[quant * 18 + kk * 2 + dd] = quant * 18 + dd * 9 + kk
    ow = f['off_w'][perm]
    obias = f['off_b'][perm]
    lhsT_off = np.transpose(ow, (1, 2, 3, 0)).reshape(P, 9, 54)

    wr = f['dcn_w'].reshape(Co, DG, Cg, 9)
    lhsT_dcn = np.transpose(wr, (1, 2, 3, 0)).reshape(P, 9, Co)

    cst = np.zeros((P, 12), dtype=np.float32)
    cst[:, 9], cst[:, 10] = MAGIC, -MAGIC
    cst[:, 0], cst[:, 1] = inv1, beta1
    cst[:, 2], cst[:, 3] = inv2, beta2 + betad
    cst[:, 4], cst[:, 5] = inv3, beta3 + inv3 * f['dcn_b']
    for kd in range(18):
        cst[kd, 6] = obias[0 * 18 + kd]   # dy bias
        cst[kd, 7] = obias[1 * 18 + kd]   # dx bias
        cst[kd, 8] = obias[2 * 18 + kd]   # mask bias

    # i-order position constants: i = ((y%2)*128 + x)*16 + y//2
    ii = np.arange(S)
    aa = ii % 16
    cc = ii // 16
    bb2 = cc // 128
    xx = cc % 128
    yloc = 2 * aa + bb2
    y_add = np.zeros((18, S), dtype=np.float32)
    x_add = np.zeros((18, S), dtype=np.float32)
    for kk in range(9):
        for dd in range(2):
            kd = 2 * kk + dd
            y_add[kd] = yloc + (kk // 3) + 1.5
            x_add[kd] = xx + (kk % 3) + 1.5

    return dict(
        lhsT1=_f(lhsT1), lhsT2=_bf(lhsT2), lhsT_sc=_f(lhsT_sc),
        lhsT_off=_bf(lhsT_off), lhsT_dcn=_bf(lhsT_dcn),
        consts=_f(cst), y_add=_f(y_add), x_add=_f(x_add), x=f['x'])


def kernel(**inputs):
    cfg = _prep(inputs)
    x = cfg.pop('x')
    B = x.shape[0]

    if 'nc' not in _CACHE:
        _CACHE['nc'] = build_nc()
    nc = _CACHE['nc']

    in_maps = []
    for cid in range(8):
        b, q = cid // 4, cid % 4
        h0 = 32 * q
        xp = np.zeros((Ci, XR, XC), dtype=np.float32)
        r_lo = 2 * h0 - 9
        s_lo, s_hi = max(r_lo, 0), min(2 * h0 + 72, 256)
        xp[:, s_lo - r_lo: s_hi - r_lo, 1:257] = x[b, :, s_lo:s_hi, :]
        rm1 = np.zeros((P, F1R), dtype=np.float32)
        for f1 in range(F1R):
            rm1[:, f1] = 1.0 if 0 <= h0 - 4 + f1 < H else 0.0
        rmf = np.zeros((P, FR), dtype=np.float32)
        for f2 in range(38):
            rmf[:, f2] = 1.0 if 0 <= h0 - 3 + f2 < H else 0.0
        m = dict(cfg)
        m['x_shard'] = np.ascontiguousarray(xp)
        m['rowmask1'] = rm1
        m['rowmaskF'] = rmf
        in_maps.append(m)

    res = run_bass_kernel_spmd(nc, in_maps, core_ids=list(range(8)))
    out = np.zeros((B, Co, H, W), dtype=np.float32)
    for cid in range(8):
        b, q = cid // 4, cid % 4
        out[b, :, 32 * q:32 * q + 32, :] = res.results[cid]['out']
    return out
